# revision 29
# baseline (speedup 1.0000x reference)
"""Bass/Tile TRN2 kernel for nn_Disen_GAT_For_Multi_Aspect (v4).

Contract: kernel(**inputs) takes FULL fp32 numpy inputs (keys as in
reference.setup_inputs()) and returns the FULL [B, A, H] fp32 output.

Strategy
--------
Data-parallel over batch B across the 8 cores (1 batch row / core, A=4
aspects per core).  The reference collapses algebraically:

  q = Wq^T asp + bq;  u = TA q; v = TB q; y = W1b v
  w[k] = sum_{i,j} q_i v_j T1[i,j,k]
  logits: ch1 = (x.Gq + Cb)/S, ch2 = (x.(Gw+Gy) + d.Gu + Cdw)/S,
          ch0 = (t.Gq + Cb)/S        (G* = Wk @ *)
  att = sum_ch comb_w[ch] * softmax_masked(logit_ch)
  att_z[h] = sum_n att_n (Wv^T x_n + bv)_h (Wv^T t_n + bv)_h

All aspect-level math (q/u/v/y/w/G, the T1 tensor contraction, the
scalar bias terms Cb/Cdw) is precomputed on the host in fp64 - it is
<1% of the FLOPs.  The device does only the stream work per aspect:

 * V matmuls (bf16): V_W = Wv^T X, V_T = Wv^T T  ([128, 512] each)
 * channel logits: ONE [3, N] PSUM accumulation group fed by three
   chains sharing tile_position (0,0): X-chain (lhsT cols
   [64Gq, 64(Gw+Gy), 0]), Dp-chain (fp8 DoubleRow, 2 K-chunks per
   instruction, cols [0, 64Gu, 0]), T-chain (cols [0, 0, 64Gq]).
   The global 64x scale keeps the fp8 Gu panel in range; the Exp
   scale 1/(64*SCALE) undoes it.  No combo matmul, no psum cast.
 * softmax: ACT Exp -> mask multiply (vector) -> z reduce ->
   reciprocal -> alpha broadcast matmul -> fused multiply-accumulate
   into attz (scalar_tensor_tensor with accum_out).

DMA staging: the 16 HW queues serve all outstanding transfers round-
robin, so consts + aspect 0 issue up front in consumption order,
aspect 1 is anchored after aspect-0 scalar ops, and aspects 2-3 are
gated by pool-buffer reuse (bufs=2) on the idle sync/gpsimd queues.
"""

import contextlib
import ctypes
import sys
import types

import numpy as np
import ml_dtypes

import concourse.bacc as bacc
import concourse.mybir as mybir
import concourse.tile as tile
from concourse.bass_utils import run_bass_kernel_spmd

B, A, N, D, H = 8, 4, 512, 1024, 128
SCALE = float(np.sqrt(H))
NCORES = 8
DC = D // H   # 8 contraction chunks of 128
GS = 64.0     # global logit scale (fp8 range), undone in the Exp
GXW = 4       # bf16 panel cols per aspect (3 used)
G8W = 8       # fp8 panel cols per aspect (3 used; 8 for 16B DR align)
GXP = GXW * 4  # bf16 panel width per chunk
G8P = G8W * 4  # fp8 panel width per chunk

F32 = mybir.dt.float32
BF16 = mybir.dt.bfloat16
F8 = mybir.dt.float8e4
BF = ml_dtypes.bfloat16
E4 = ml_dtypes.float8_e4m3fn
AF = mybir.ActivationFunctionType
OP = mybir.AluOpType
DR = mybir.MatmulPerfMode.DoubleRow

# cpackf (f32) column layout
CF_BV = 0              # bv column
CF_BA = 1              # bias_all [3 partitions, 4 cols] rows (ch1,ch2,ch0)
CF_CW = 5              # comb_w column (3 partitions, rows (c1,c2,c0))
CF_W = 6
# cpackb (bf16) column layout
CB_GX = 0                      # [128, 8, 16] X-chain panel
CB_GT = DC * GXP               # [128, 8, 16] T-chain panel
CB_M3 = 2 * DC * GXP           # [3, N] mask row replicated
CB_W = 2 * DC * GXP + N

NWARM = 3

LAST_RESULTS = None  # test harness peeks at this


def _build(ncores=NCORES):
    nc = bacc.Bacc("TRN2", target_bir_lowering=False, debug=False,
                   num_devices=ncores)

    xs = nc.dram_tensor("xs", [A, 128, 2, DC, N], BF16, kind="ExternalInput")
    dp8 = nc.dram_tensor("dp8", [A, 128, DC, N], F8, kind="ExternalInput")
    cpackf = nc.dram_tensor("cpackf", [128, CF_W], F32, kind="ExternalInput")
    wvpk = nc.dram_tensor("wvpk", [128, DC * H], BF16, kind="ExternalInput")
    cpackb = nc.dram_tensor("cpackb", [128, CB_W], BF16, kind="ExternalInput")
    gal8 = nc.dram_tensor("gal8", [128, DC * G8P], F8, kind="ExternalInput")
    out = nc.dram_tensor("out", [H, A], F32, kind="ExternalOutput")

    esc = 1.0 / (GS * SCALE)

    with tile.TileContext(nc) as tc:
        with (
            tc.tile_pool(name="const", bufs=1) as cp,
            tc.tile_pool(name="xzone", bufs=2) as xp,
            tc.tile_pool(name="work", bufs=2) as wp,
            tc.tile_pool(name="vzone", bufs=4, space="PSUM") as vps,
            tc.tile_pool(name="rzone", bufs=2, space="PSUM") as rps,
            tc.tile_pool(name="szone", bufs=2, space="PSUM") as sps,
        ):
            # ---- PE warm-up: opens the clock gate before real work ----
            wuc = cp.tile([128, 1], BF16, tag="wuc")
            nc.vector.memset(wuc, 1.0)
            wub = cp.tile([128, N], BF16, tag="wub")
            nc.vector.memset(wub, 1.0)
            ps_wu = sps.tile([1, N], F32, tag="s")
            for i in range(NWARM):
                nc.tensor.matmul(ps_wu, lhsT=wuc, rhs=wub,
                                 start=(i == 0), stop=(i == NWARM - 1))

            # ---- tiles ------------------------------------------------
            wvs = cp.tile([128, DC, H], BF16, tag="wvs")
            cpf = cp.tile([128, CF_W], F32, tag="cpf")
            cpb = cp.tile([128, CB_W], BF16, tag="cpb")
            g8 = cp.tile([128, DC, G8P], F8, tag="g8")
            xx_t, xt_t, dp_t = {}, {}, {}
            for a in range(A):
                xx = xp.tile([128, DC, N], BF16, tag="xx")
                dpa = xp.tile([128, DC, N], F8, tag="dp")
                xt_ = xp.tile([128, DC, N], BF16, tag="xt")
                xx_t[a], xt_t[a], dp_t[a] = xx, xt_, dpa

            # ---- DMA staging (consumption order; see module doc) ------
            nc.sync.dma_start(out=xx_t[0][:, 0:2], in_=xs.ap()[0, :, 0, 0:2])
            nc.sync.dma_start(out=wvs.rearrange("p c h -> p (c h)"),
                              in_=wvpk.ap())
            nc.sync.dma_start(out=xx_t[0][:, 2:4], in_=xs.ap()[0, :, 0, 2:4])
            nc.sync.dma_start(out=xx_t[0][:, 4:8], in_=xs.ap()[0, :, 0, 4:8])
            nc.sync.dma_start(out=xt_t[0][:, 0:DC // 2],
                              in_=xs.ap()[0, :, 1, 0:DC // 2])
            nc.scalar.dma_start(out=cpf, in_=cpackf.ap())
            nc.scalar.dma_start(out=cpb, in_=cpackb.ap())
            nc.scalar.dma_start(out=xt_t[0][:, DC // 2:DC],
                                in_=xs.ap()[0, :, 1, DC // 2:DC])
            nc.gpsimd.dma_start(out=g8.rearrange("p c g -> p (c g)"),
                                in_=gal8.ap())
            nc.gpsimd.dma_start(out=dp_t[0], in_=dp8.ap()[0])
            # aspect 1 fetches are anchored on aspect-0 scalar compute;
            # aspects 2-3 are emitted at the end of aspects 0/1 on the
            # idle sync/gpsimd queues, gated by pool-buffer WAR reuse

            def fetch_x(a, eng=None):
                (eng or nc.scalar).dma_start(out=xx_t[a],
                                             in_=xs.ap()[a, :, 0])

            def fetch_dt(a, eng=None):
                (eng or nc.scalar).dma_start(out=dp_t[a], in_=dp8.ap()[a])
                (eng or nc.scalar).dma_start(out=xt_t[a],
                                             in_=xs.ap()[a, :, 1])

            # ---- constant views ---------------------------------------
            gx_v = cpb[:, CB_GX:CB_GX + DC * GXP].rearrange(
                "p (c g) -> p c g", c=DC)
            gt_v = cpb[:, CB_GT:CB_GT + DC * GXP].rearrange(
                "p (c g) -> p c g", c=DC)
            m3 = cpb[0:3, CB_M3:CB_M3 + N]
            bv_c = cpf[:, CF_BV:CF_BV + 1]
            bias_all = cpf[0:3, CF_BA:CF_BA + A]
            combw3 = cpf[0:3, CF_CW:CF_CW + 1]

            ones3r = cp.tile([3, 128], BF16, tag="ones3r")
            nc.vector.memset(ones3r, 1.0)
            attz = cp.tile([H, A], F32, tag="attz")

            # ---- per-aspect stream work -------------------------------
            # Uniform-config matmul chains run at 216ns/instr; alternating
            # PE tile configs cost ~+105ns each, so chains stay contiguous.
            # Aspects 0-2: V_W first (only needs the X plane, which lands
            # first).  Last aspect: rows first so the softmax chain hides
            # completely under the V_W/V_T chains and the tail is short.
            for a in range(A):
                xx, xt_, da = xx_t[a], xt_t[a], dp_t[a]
                w3 = slice(GXW * a, GXW * a + 3)
                w8 = slice(G8W * a, G8W * a + 3)
                last = a == A - 1

                ps_vw = vps.tile([H, N], F32, tag="v")
                ps_ch = rps.tile([3, N], F32, tag="ch")
                vvw = wp.tile([H, N], BF16, tag="vvw")
                e3 = wp.tile([3, N], BF16, tag="e3")
                z3 = wp.tile([3, 1], F32, tag="z3")

                def vw_chain():
                    for c in range(DC):
                        nc.tensor.matmul(ps_vw, lhsT=wvs[:, c, :],
                                         rhs=xx[:, c, :], start=(c == 0),
                                         stop=(c == DC - 1))
                    nc.scalar.activation(vvw, ps_vw, AF.Identity, bias=bv_c)
                    if a == 0:
                        fetch_x(1)

                def rows_chains():
                    # one [3, N] accumulation group: X cols, Dp (fp8
                    # DoubleRow), T cols; all tile_position (0,0)
                    for c in range(DC):
                        nc.tensor.matmul(ps_ch, lhsT=gx_v[:, c, w3],
                                         rhs=xx[:, c, :], start=(c == 0),
                                         stop=False)
                    for c2 in range(DC // 2):
                        nc.tensor.matmul(ps_ch,
                                         lhsT=g8[:, 2 * c2:2 * c2 + 2, w8],
                                         rhs=da[:, 2 * c2:2 * c2 + 2, :],
                                         start=False, stop=False,
                                         perf_mode=DR)
                    for c in range(DC):
                        nc.tensor.matmul(ps_ch, lhsT=gt_v[:, c, w3],
                                         rhs=xt_[:, c, :], start=False,
                                         stop=(c == DC - 1))
                    # exp with the bias fold; mask + z on vector after
                    nc.scalar.activation(e3, ps_ch, AF.Exp,
                                         bias=bias_all[:, a:a + 1], scale=esc)
                    if a == 0:
                        fetch_dt(1)

                ps_vt = vps.tile([H, N], F32, tag="v")

                def vt_chain():
                    for c in range(DC):
                        nc.tensor.matmul(ps_vt, lhsT=wvs[:, c, :],
                                         rhs=xt_[:, c, :], start=(c == 0),
                                         stop=(c == DC - 1))

                if last:
                    rows_chains()
                    vw_chain()
                    vt_chain()
                else:
                    vw_chain()
                    rows_chains()
                    vt_chain()

                # softmax epilogue on vector (overlaps the V_T chain)
                e3m = wp.tile([3, N], BF16, tag="e3m")
                nc.vector.tensor_tensor(e3m, e3, m3, op=OP.mult)
                nc.vector.tensor_reduce(z3, e3m, axis=mybir.AxisListType.X,
                                        op=OP.add)
                rz = wp.tile([3, 1], F32, tag="rz")
                nc.vector.reciprocal(rz, z3)
                alpha = wp.tile([3, 1], F32, tag="alpha")
                nc.vector.tensor_mul(alpha, rz, combw3)
                arep = wp.tile([3, H], BF16, tag="arep")
                nc.vector.tensor_scalar_mul(arep, ones3r, alpha)
                # pprod = (VT+bv)*(VW+bv) right after the VT chain, then
                # attMM, then one multiply-accumulate into attz[:, a]
                pprod = wp.tile([H, N], BF16, tag="pprod")
                nc.vector.scalar_tensor_tensor(
                    pprod, ps_vt, bv_c, vvw, op0=OP.add, op1=OP.mult)
                ps_att = sps.tile([H, N], F32, tag="s")
                nc.tensor.matmul(ps_att, lhsT=arep, rhs=e3m,
                                 start=True, stop=True)
                junk = wp.tile([H, N], BF16, tag="junk")
                nc.vector.scalar_tensor_tensor(
                    junk, ps_att, 1.0, pprod, op0=OP.mult, op1=OP.mult,
                    accum_out=attz[:, a:a + 1])

                if a + 2 < A:  # WAR-gated prefetch of aspect a+2
                    fetch_x(a + 2, nc.sync)
                    fetch_dt(a + 2, nc.gpsimd)
                # per-aspect output column on the idle gpsimd queue so the
                # final transfer only waits on the last aspect's 512B
                nc.gpsimd.dma_start(out=out.ap()[:, a:a + 1],
                                    in_=attz[:, a:a + 1])

    nc.compile()
    return nc


def _host_precompute(f):
    """All aspect-level math in fp64 on host -> per-core const packs."""
    S = SCALE
    Wq = f["Wq"].astype(np.float64)
    Wk = f["Wk"].astype(np.float64)
    TA = f["trans_W"][:H].astype(np.float64)   # [H, H]
    TB = f["trans_W"][H:].astype(np.float64)
    W1a = f["W1_W"][:H].astype(np.float64)
    W1b = f["W1_W"][H:].astype(np.float64)
    T1 = f["T1"].astype(np.float64)
    bq, bk = f["bq"].astype(np.float64), f["bk"].astype(np.float64)
    W1_b = f["W1_b"].astype(np.float64)
    trans_b = f["trans_b"].astype(np.float64)

    asp = f["aspect_feature"].astype(np.float64)          # [B, A, D]
    q = asp @ Wq + bq                                     # [B, A, H]
    u = np.einsum("kh,bah->bak", TA, q)
    v = np.einsum("jh,bah->baj", TB, q)
    y = np.einsum("kj,baj->bak", W1b, v)
    a3 = np.einsum("ij,bai->baj", W1a, q)
    QT = np.einsum("bai,ijk->bajk", q, T1)
    w = np.einsum("bajk,baj->bak", QT, v)
    # stream panels: ch1 <- x.Gq, ch2 <- x.G(w+y) + d.Gu, ch0 <- t.Gq
    Gq = np.einsum("dh,bah->bad", Wk, q)
    Gc2 = np.einsum("dh,bah->bad", Wk, w + y)
    Gu = np.einsum("dh,bah->bad", Wk, u)
    Cb = q @ bk                                           # [B, A]
    Cdw = ((u + w + y) @ bk + ((a3 + W1_b) * v).sum(-1) + q @ trans_b)
    bias_all = np.stack([Cb, Cdw, Cb], axis=1) / S        # [B, 3, A]
    return Gq, Gc2, Gu, bias_all


def _prep_inputs(inputs):
    f = {k: np.asarray(v, dtype=np.float32) for k, v in inputs.items()}
    Gq, Gc2, Gu, bias_all = _host_precompute(f)

    wvpk = np.transpose(
        f["Wv"].reshape(DC, 128, H), (1, 0, 2)).reshape(128, DC * H)
    cw = f["comb_w"]

    in_maps = []
    for b in range(NCORES):
        gx = np.zeros((D, GXP), np.float64)
        gt = np.zeros((D, GXP), np.float64)
        g8 = np.zeros((D, G8P), np.float64)
        for a in range(A):
            gx[:, GXW * a + 0] = GS * Gq[b, a]
            gx[:, GXW * a + 1] = GS * Gc2[b, a]
            gt[:, GXW * a + 2] = GS * Gq[b, a]
            g8[:, G8W * a + 1] = GS * Gu[b, a]
        cb = np.zeros((128, CB_W), np.float32)
        cb[:, CB_GX:CB_GX + DC * GXP] = gx.reshape(
            DC, 128, GXP).transpose(1, 0, 2).reshape(128, -1)
        cb[:, CB_GT:CB_GT + DC * GXP] = gt.reshape(
            DC, 128, GXP).transpose(1, 0, 2).reshape(128, -1)
        cb[0:3, CB_M3:CB_M3 + N] = np.tile(f["fmask"][b], (3, 1))

        cf = np.zeros((128, CF_W), np.float32)
        cf[:, CF_BV] = f["bv"]
        cf[0:3, CF_BA:CF_BA + A] = bias_all[b]
        cf[0:3, CF_CW] = [cw[1], cw[2], cw[0]]

        m = {
            "cpackb": cb.astype(BF),
            "wvpk": wvpk.astype(BF),
            "cpackf": cf,
            "gal8": np.clip(
                g8.reshape(DC, 128, G8P).transpose(1, 0, 2)
                  .reshape(128, -1), -448, 448).astype(E4),
        }
        xst = np.stack([f["feature"][b], f["all_type_feature"][b]], axis=1)
        # [A, 2, N, D] -> [A, 128(p), 2(s), DC(c), N]
        m["xs"] = np.ascontiguousarray(
            xst.transpose(0, 1, 3, 2).reshape(A, 2, DC, 128, N)
               .transpose(0, 3, 1, 2, 4)).astype(BF)
        dpt = f["dep_feature"][b].transpose(0, 2, 1).reshape(A, DC, 128, N)
        m["dp8"] = np.clip(np.ascontiguousarray(dpt.transpose(0, 2, 1, 3)),
                           -240, 240).astype(E4)
        in_maps.append(m)
    return in_maps


def _install_ntff_shim():
    """Provide antenv.axon_hooks (absent in this image) so trace=True can
    drive NTFF capture through libaxon_pjrt.so."""
    if "antenv.axon_hooks" in sys.modules:
        return
    import antenv

    mod = types.ModuleType("antenv.axon_hooks")
    mod._hook = None
    mod.set_axon_ntff_profile_hook = lambda h: setattr(mod, "_hook", h)
    mod.get_axon_ntff_profile_hook = lambda: mod._hook
    sys.modules["antenv.axon_hooks"] = mod
    antenv.axon_hooks = mod

    so_path = "/opt/axon/libaxon_pjrt.so"
    try:
        lib = ctypes.CDLL(so_path)
    except OSError:
        return
    if not hasattr(lib, "axon_start_nrt_profile"):
        return
    lib.axon_start_nrt_profile.argtypes = [ctypes.POINTER(ctypes.c_int64),
                                           ctypes.c_size_t]
    lib.axon_start_nrt_profile.restype = ctypes.c_int64
    lib.axon_stop_nrt_profile.argtypes = [ctypes.c_char_p]
    lib.axon_stop_nrt_profile.restype = ctypes.c_int64

    @contextlib.contextmanager
    def _hook(output_dir, device_ids):
        import jax

        jax.devices()
        if device_ids:
            ids = (ctypes.c_int64 * len(device_ids))(*device_ids)
            rc = lib.axon_start_nrt_profile(ids, len(device_ids))
        else:
            rc = lib.axon_start_nrt_profile(None, 0)
        if rc != 0:
            raise RuntimeError(f"axon_start_nrt_profile rc={rc}")
        try:
            yield
        finally:
            n = lib.axon_stop_nrt_profile(str(output_dir).encode())
            print(f"profile: {n} file(s) written to {output_dir}")

    mod.set_axon_ntff_profile_hook(_hook)


def kernel(feature, dep_feature, aspect_feature, all_type_feature, fmask,
           Wq, bq, Wk, bk, Wv, bv, trans_W, trans_b, T1, W1_W, W1_b, comb_w,
           _profile=False, _tmpdir=None):
    global LAST_RESULTS
    inputs = dict(feature=feature, dep_feature=dep_feature,
                  aspect_feature=aspect_feature,
                  all_type_feature=all_type_feature, fmask=fmask, Wq=Wq,
                  bq=bq, Wk=Wk, bk=bk, Wv=Wv, bv=bv, trans_W=trans_W,
                  trans_b=trans_b, T1=T1, W1_W=W1_W, W1_b=W1_b,
                  comb_w=comb_w)
    nc = _build()
    in_maps = _prep_inputs(inputs)
    if _profile:
        _install_ntff_shim()
    res = run_bass_kernel_spmd(nc, in_maps, list(range(NCORES)),
                               trace=_profile, tmpdir=_tmpdir)
    LAST_RESULTS = res
    full = np.stack([res.results[c]["out"].T for c in range(NCORES)])
    return full.astype(np.float32)
